# revision 42
# baseline (speedup 1.0000x reference)
"""Trainium2 Bass kernel for nn_DiffAttn (differential attention).

Reference computation (per batch b):
    Q = X @ Wq.T + bq ; K = X @ Wk.T + bk ; V = X @ Wv.T + bv
    Q1,Q2 / K1,K2 = halves of feature dim
    A_j = (Q_j @ K_j.T) / sqrt(DIM)
    out = softmax(A1) @ V - scalar * softmax(A2) @ V

Sharding: 8 cores = 4 batches x 2 halves. Core (b,h) owns query rows
[h*1024,(h+1)*1024) of batch b AND computes the K^T/V projections only
for the SAME row block as keys; the two pair-mates exchange their K/V
halves with an intra-pair AllGather (replica groups {0,1},{2,3},...)
so neither projects the full sequence. Output slabs are disjoint.

Device-side layouts avoid all on-chip transposes: the host pre-transposes
X^T and W^T so every matmul contraction lands on SBUF partitions.
Q^T/K^T are produced in fp8e4 (scaled by QKS=4) directly by the
projection epilogue and the score matmuls run as fp8 DoubleRow (K=256
per instruction, 2x FLOP rate); P=exp and V are bf16 and the single
attn@V GEMM runs in bf16. Attention weights are normalized BEFORE the
V matmul (A = P1/r1 - scalar*P2/r2); row sums come from an all-ones
stationary matmul replicated across partitions, and 1/r is computed as
exp(-ln r) on the Scalar engine.

Overlap notes (measured on trn2, ~210 us HW exec, rel-err ~1.5e-2):
each AllGather is split in two chunks triggered mid-projection so the
~30 us mesh latency hides under the remaining phase-1 matmuls; both
score chunks run before either attn@V chunk so the serial
normalization tail overlaps score matmuls; weight DMAs are issued in
phase-consumption order (wk, wv, wq). Moving the row-sum accumulation
to GpSimd was tried and REGRESSED (gpsimd tensor ops ~1.6 us per
[128,512] tile, slower than the exp cadence).
"""

import json
import math
import os
from contextlib import ExitStack

import numpy as np
import ml_dtypes

import concourse.bass as bass
import concourse.tile as tile
from concourse import mybir
from concourse.bass_utils import run_bass_kernel_spmd


def _split_waits(raw: bytes, max_waits: int = 1) -> bytes:
    """walrus's CoreV3 codegen rejects instructions carrying more than one
    sync wait ("Too many sync wait commands"); Tile's kernel-tail drain
    aggregates one wait per live processor. Hoist excess waits onto chained
    same-engine Drain instructions inserted immediately before the offender."""
    m = json.loads(raw)
    uid = 0
    for fn in m["functions"]:
        for blk in fn["blocks"]:
            out = []
            for ins in blk["instructions"]:
                sy = ins.get("sync_info") or {}
                waits = sy.get("on_wait") or []
                if len(waits) > max_waits:
                    head, keep = waits[:-max_waits], waits[-max_waits:]
                    while head:
                        chunk, head = head[:max_waits], head[max_waits:]
                        uid += 1
                        out.append(
                            {
                                "engine": ins["engine"],
                                "ins": [],
                                "is_reset_sema": False,
                                "name": f"{ins['name']}-wsplit{uid}",
                                "opcode": "Drain",
                                "outs": [],
                                "sync_info": {"on_update": [], "on_wait": chunk},
                            }
                        )
                    sy["on_wait"] = keep
                out.append(ins)
            blk["instructions"] = out
    return json.dumps(m).encode()

B, S, DIM = 4, 2048, 1024
H = DIM // 2
NCORES = 8
QLEN = S // 2          # queries (and locally-projected keys) per core
SCALE = 1.0 / math.sqrt(DIM)

BF16 = mybir.dt.bfloat16
F32 = mybir.dt.float32
F32R = mybir.dt.float32r
FP8 = mybir.dt.float8e4
DR = mybir.MatmulPerfMode.DoubleRow

DT = DIM // 128        # 8  contraction tiles over model dim
CT = DIM // 128        # 8  feature tiles of Q^T/K^T
KT = S // 128          # 16 key tiles
KTL = QLEN // 128      # 8  local key tiles
NQC = QLEN // 512      # 2  query chunks of 512
VW = DIM              # V width (row sums come from an ones-row matmul instead)
QKS = 4.0             # fp8 pre-scale on Q^T/K^T (folded out of the exp scale)

# test harness hooks (the grader never touches these)
TRACE = False
LAST_RESULTS = None


def _build_bass():
    nc = bass.Bass(
        trn_type="TRN2",
        target_bir_lowering=False,
        debug=False,
        num_devices=NCORES,
    )

    xtq = nc.dram_tensor("xtq", [DIM, QLEN], BF16, kind="ExternalInput")
    wqt = nc.dram_tensor("wqt", [DIM, DIM], BF16, kind="ExternalInput")
    wkt = nc.dram_tensor("wkt", [DIM, DIM], BF16, kind="ExternalInput")
    wvt = nc.dram_tensor("wvt", [DIM, DIM], BF16, kind="ExternalInput")
    bqr = nc.dram_tensor("bqr", [128, CT], F32, kind="ExternalInput")
    bkr = nc.dram_tensor("bkr", [128, CT], F32, kind="ExternalInput")
    bvb = nc.dram_tensor("bvb", [128, DIM], F32, kind="ExternalInput")
    scv = nc.dram_tensor("scv", [128, 1], F32, kind="ExternalInput")
    outp = nc.dram_tensor("out", [QLEN, DIM], F32, kind="ExternalOutput")

    Id = mybir.ActivationFunctionType.Identity
    Exp = mybir.ActivationFunctionType.Exp
    PAIRS = [[0, 1], [2, 3], [4, 5], [6, 7]]

    with tile.TileContext(nc) as tc, ExitStack() as ctx:
        const = ctx.enter_context(tc.tile_pool(name="const", bufs=1))
        persist = ctx.enter_context(tc.tile_pool(name="persist", bufs=1))
        dram = ctx.enter_context(tc.tile_pool(name="dram", bufs=1, space="DRAM"))
        ps_s = ctx.enter_context(
            tc.tile_pool(name="ps_s", bufs=3, space="PSUM")
        )

        bq_sb = const.tile([128, CT], F32)
        nc.sync.dma_start(out=bq_sb[:, :], in_=bqr[:, :])
        bk_sb = const.tile([128, CT], F32)
        nc.sync.dma_start(out=bk_sb[:, :], in_=bkr[:, :])
        sc_sb = const.tile([128, 1], F32)
        nc.sync.dma_start(out=sc_sb[:, :], in_=scv[:, :])
        ones_sb = const.tile([128, 2], F32)
        nc.vector.memset(ones_sb[:, :], 1.0)

        # Warm the PE clock gate (HAM) during the initial input-DMA wait:
        # a chain of tiny dependent matmuls gives sustained PE activity so
        # the first projection matmuls run at 2.4 GHz, not 1.2.
        with tc.psum_pool(name="ps_w", bufs=1) as ps_w:
            warm = ps_w.tile([2, 2], F32, name="warm")
            for _ in range(24):
                nc.tensor.matmul(
                    warm[:, :], ones_sb[:, :], ones_sb[:, :], start=True, stop=True
                )

        # persistent products of the projection phase.
        # Q^T/K^T are fp8e4 (scaled by QKS) in DoubleRow-paired tiles:
        # tile t holds feature rows (256t..256t+128) in slot 0 and
        # (256t+128..256t+256) in slot 1 so a [128, 2, n] slice is a K=256
        # fp8 DoubleRow operand. V is bf16.
        q_sb = [persist.tile([128, 2, QLEN], FP8, name=f"q{i}") for i in range(CT // 2)]
        k_sb = [persist.tile([128, 2, S], FP8, name=f"k{i}") for i in range(CT // 2)]
        v_sb = [persist.tile([128, VW], BF16, name=f"v{i}") for i in range(KT)]

        # AllGather bounce buffers (flat concat along the leading axis:
        # out.reshape(2, ...)[g] = rank g's input within the pair).
        # Both gathers are split in two so the wire transfer starts while
        # the second half of the projection is still running.
        kag_in = [
            dram.tile([2, 128, 2, QLEN], FP8, name=f"kag_in{i}") for i in range(2)
        ]
        kag_out = [
            dram.tile([2, 2, 128, 2, QLEN], FP8, name=f"kag_out{i}") for i in range(2)
        ]
        vag_in = [
            dram.tile([KTL // 2, 128, VW], BF16, name=f"vag_in{i}") for i in range(2)
        ]
        vag_out = [
            dram.tile([2, KTL // 2, 128, VW], BF16, name=f"vag_out{i}")
            for i in range(2)
        ]

        # X^T tiles for this core's row block: queries for Q^T, and the
        # same rows serve as the locally-projected keys for K^T/V.
        xtp = tc.alloc_tile_pool(name="xtp", bufs=1)
        xq_t = [xtp.tile([128, QLEN], BF16, name=f"xq{d}") for d in range(DT)]

        # weight pools: wk / wv / wq — DMAs issued in phase-consumption order
        # so the DMA queues deliver each phase's operands just ahead of its
        # matmuls (wv before wq: phase 1b precedes 1c).
        wkp = tc.alloc_tile_pool(name="wk", bufs=1)
        wvp = tc.alloc_tile_pool(name="wv", bufs=1)
        wqp = tc.alloc_tile_pool(name="wq", bufs=1)
        wk_t = [wkp.tile([128, DIM], BF16, name=f"wk{d}") for d in range(DT)]
        wv_t = [wvp.tile([128, DIM], BF16, name=f"wv{d}") for d in range(DT)]
        bv_sb = wvp.tile([128, DIM], F32, name="bv_sb")
        wq_t = [wqp.tile([128, DIM], BF16, name=f"wq{d}") for d in range(DT)]
        # First-wave DMAs are column-halved so the opening Kproj block
        # (c 0..3, n=0) only waits on 2MB (xq/wk first halves), not 4MB.
        for d in range(DT):
            nc.sync.dma_start(
                out=xq_t[d][:, :512], in_=xtq[d * 128 : (d + 1) * 128, :512]
            )
            nc.sync.dma_start(
                out=wk_t[d][:, :128], in_=wkt[d * 128 : (d + 1) * 128, :128]
            )
        for d in range(DT):
            nc.sync.dma_start(
                out=wk_t[d][:, 128:512], in_=wkt[d * 128 : (d + 1) * 128, 128:512]
            )
        for d in range(DT):
            nc.sync.dma_start(
                out=xq_t[d][:, 512:], in_=xtq[d * 128 : (d + 1) * 128, 512:]
            )
        for d in range(DT):
            nc.sync.dma_start(
                out=wk_t[d][:, 512:], in_=wkt[d * 128 : (d + 1) * 128, 512:]
            )
        nc.sync.dma_start(out=bv_sb[:, :], in_=bvb[:, :])
        for d in range(DT):
            nc.sync.dma_start(out=wv_t[d][:, :], in_=wvt[d * 128 : (d + 1) * 128, :])
        for d in range(DT):
            nc.sync.dma_start(out=wq_t[d][:, :], in_=wqt[d * 128 : (d + 1) * 128, :])

        # ---- Phase 1a: local K^T half = Wk^T.T @ X^T[:, own rows]  (+bk) ----
        # The pair exchange is split in two chunks (feature tiles {0,1} then
        # {2,3}), each triggered as soon as its projection slice is done, so
        # the mesh latency overlaps the rest of phase 1.
        kh = [persist.tile([128, 2, QLEN], FP8, name=f"kh{i}") for i in range(CT // 2)]
        with nc.named_scope("proj_k"):
            # (c-group, n) block order so the first block only touches the
            # first-wave DMA halves; each c-group's AllGather chunk fires as
            # soon as both its n chunks are done.
            korder = [
                (cg, n) for cg in range(2) for n in range(QLEN // 512)
            ]
            for cg, n in korder:
                for c in range(4 * cg, 4 * cg + 4):
                    ps = ps_s.tile([128, 512], F32, tag="ps", name="psk")
                    for d in range(DT):
                        nc.tensor.matmul(
                            ps[:, :],
                            wk_t[d][:, c * 128 : (c + 1) * 128],
                            xq_t[d][:, n * 512 : (n + 1) * 512],
                            start=(d == 0),
                            stop=(d == DT - 1),
                        )
                    nc.scalar.activation(
                        kh[c // 2][:, c % 2, n * 512 : (n + 1) * 512],
                        ps[:, :],
                        Id,
                        scale=QKS,
                        bias=bk_sb[:, c : c + 1],
                    )
                if n == QLEN // 512 - 1:
                    ch = cg
                    for tt in range(2):
                        t = 2 * ch + tt
                        nc.sync.dma_start(
                            out=kag_in[ch][tt, :, :, :], in_=kh[t][:, :, :]
                        )
                    nc.gpsimd.collective_compute(
                        "AllGather",
                        mybir.AluOpType.bypass,
                        replica_groups=PAIRS,
                        ins=[kag_in[ch].opt()],
                        outs=[kag_out[ch].opt()],
                    )
                    for hh in range(2):
                        for tt in range(2):
                            t = 2 * ch + tt
                            nc.sync.dma_start(
                                out=k_sb[t][:, :, hh * QLEN : (hh + 1) * QLEN],
                                in_=kag_out[ch][hh, tt, :, :, :],
                            )

        # ---- Phase 1b: Q^T = Wq^T.T @ X^T[:, own rows]  (+bq) ----
        with nc.named_scope("proj_q"):
            for c in range(CT):
                for n in range(QLEN // 512):
                    ps = ps_s.tile([128, 512], F32, tag="ps", name="psq")
                    for d in range(DT):
                        nc.tensor.matmul(
                            ps[:, :],
                            wq_t[d][:, c * 128 : (c + 1) * 128],
                            xq_t[d][:, n * 512 : (n + 1) * 512],
                            start=(d == 0),
                            stop=(d == DT - 1),
                        )
                    nc.scalar.activation(
                        q_sb[c // 2][:, c % 2, n * 512 : (n + 1) * 512],
                        ps[:, :],
                        Id,
                        scale=QKS,
                        bias=bq_sb[:, c : c + 1],
                    )

        # ---- Phase 1c: local V half = X^T[:, own rows].T @ Wv^T (+bv) ----
        vh = [persist.tile([128, VW], BF16, name=f"vh{i}") for i in range(KTL)]
        with nc.named_scope("proj_v"):
            for k in range(KTL):
                for n in range(DIM // 512):
                    ps = ps_s.tile([128, 512], F32, tag="ps", name="psv")
                    for d in range(DT):
                        nc.tensor.matmul(
                            ps[:, :],
                            xq_t[d][:, k * 128 : (k + 1) * 128],
                            wv_t[d][:, n * 512 : (n + 1) * 512],
                            start=(d == 0),
                            stop=(d == DT - 1),
                        )
                    nc.vector.tensor_add(
                        vh[k][:, n * 512 : (n + 1) * 512],
                        ps[:, :],
                        bv_sb[:, n * 512 : (n + 1) * 512],
                    )
                if k % 4 == 3:
                    ch = k // 4
                    for kk in range(4):
                        nc.sync.dma_start(
                            out=vag_in[ch][kk, :, :], in_=vh[4 * ch + kk][:, :]
                        )
                    nc.gpsimd.collective_compute(
                        "AllGather",
                        mybir.AluOpType.bypass,
                        replica_groups=PAIRS,
                        ins=[vag_in[ch].opt()],
                        outs=[vag_out[ch].opt()],
                    )
                    for hh in range(2):
                        for kk in range(4):
                            nc.sync.dma_start(
                                out=v_sb[hh * KTL + 4 * ch + kk][:, :],
                                in_=vag_out[ch][hh, kk, :, :],
                            )

        wqp.release()
        wvp.release()
        wkp.release()
        xtp.release()

        # ---- Phase 2: attention, one 512-query chunk at a time ----
        # Normalize P before the V matmul so only ONE attn@V GEMM is needed:
        #   A^T = P1^T * bcast(1/r1) - P2^T * bcast(scalar/r2);  out = A^T.T @ V
        # r_j comes from an ones-row stationary matmul (column sums of P^T);
        # bcast replicates the [1, q] reciprocal row across partitions via a
        # K=1 ones-column matmul.
        lnsc_sb = const.tile([128, 1], F32)
        nc.scalar.activation(lnsc_sb[:, :], sc_sb[:, :], mybir.ActivationFunctionType.Ln)
        ones_sq = const.tile([128, 128], BF16)
        nc.vector.memset(ones_sq[:, :], 1.0)

        with (
            tc.tile_pool(name="pP", bufs=1) as pP,
            tc.tile_pool(name="ps_r", bufs=2, space="PSUM") as ps_r,
            tc.tile_pool(name="ps_u", bufs=3, space="PSUM") as ps_u,
            tc.tile_pool(name="small", bufs=4) as small,
            tc.tile_pool(name="tmp2", bufs=2) as tmp2,
            tc.tile_pool(name="ostage", bufs=3) as ostage,
        ):
            p_sb = [
                [
                    [pP.tile([128, 512], BF16, name=f"p{qc}_{j}_{k}") for k in range(KT)]
                    for j in range(2)
                ]
                for qc in range(NQC)
            ]
            # Both scores chunks run back-to-back on the PE so the serial
            # normalization tail (rowsum -> ln -> exp -> mul/sub) of chunk
            # qc overlaps the next chunk's score matmuls; attn@V follows.
            for qc in range(NQC):
                # scores S^T[k, q] = K_j^T.T @ Q_j^T via fp8 DoubleRow (K=512
                # in 2 accumulating K=256 MMs); P = exp(s*S^T); r = col sums
                bcs = []
                scope_s = nc.enter_named_scope(f"attn_s{qc}", False)
                for j in range(2):
                    # r replicated across partitions: ones[128,128].T @ P = col sums
                    r_ps = ps_r.tile([128, 512], F32, tag="r", name=f"r{j}")
                    for k in range(KT):
                        ps = ps_s.tile([128, 512], F32, tag="ps", name="pss")
                        for ti in range(2):
                            t = 2 * j + ti
                            nc.tensor.matmul(
                                ps[:, :],
                                k_sb[t][:, :, k * 128 : (k + 1) * 128],
                                q_sb[t][:, :, qc * 512 : (qc + 1) * 512],
                                start=(ti == 0),
                                stop=(ti == 1),
                                perf_mode=DR,
                            )
                        nc.scalar.activation(
                            p_sb[qc][j][k][:, :], ps[:, :], Exp, scale=SCALE / (QKS * QKS)
                        )
                        nc.tensor.matmul(
                            r_ps[:, :],
                            ones_sq[:, :],
                            p_sb[qc][j][k][:, :],
                            start=(k == 0),
                            stop=(k == KT - 1),
                        )
                    # bc_j = exp(-ln r_j) = 1/r_j on the Scalar engine
                    # (j=1 folds the input scalar in via a +ln(scalar) bias)
                    lnr = tmp2.tile([128, 512], F32, tag="lnr", name="lnr")
                    nc.scalar.activation(
                        lnr[:, :], r_ps[:, :], mybir.ActivationFunctionType.Ln
                    )
                    bc = small.tile([128, 512], F32, tag=f"bc{j}", name=f"bc{j}")
                    if j == 0:
                        nc.scalar.activation(bc[:, :], lnr[:, :], Exp, scale=-1.0)
                    else:
                        nc.scalar.activation(
                            bc[:, :], lnr[:, :], Exp, scale=-1.0, bias=lnsc_sb[:, :]
                        )
                    bcs.append(bc)
                    if j == 0:
                        # P1 *= 1/r1 immediately — overlaps the j=1 scores
                        for k in range(KT):
                            nc.vector.tensor_mul(
                                p_sb[qc][0][k][:, :], p_sb[qc][0][k][:, :], bc[:, :]
                            )
                nc.leave_named_scope(f"attn_s{qc}", scope_s[0], False)

                # A^T[k] = P1[k] - P2[k]*bc2s  (in place into p_sb[qc][1])
                scope_a = nc.enter_named_scope(f"attn_a{qc}", False)
                for k in range(KT):
                    nc.vector.tensor_mul(
                        p_sb[qc][1][k][:, :], p_sb[qc][1][k][:, :], bcs[1][:, :]
                    )
                    nc.vector.tensor_sub(
                        p_sb[qc][1][k][:, :], p_sb[qc][0][k][:, :], p_sb[qc][1][k][:, :]
                    )
                nc.leave_named_scope(f"attn_a{qc}", scope_a[0], False)

            # out rows = A^T.T @ V (both chunks)
            for qc in range(NQC):
                scope_u = nc.enter_named_scope(f"attn_u{qc}", False)
                for t in range(4):
                    row = qc * 512 + t * 128
                    for n in range(DIM // 512):
                        lo, hi = n * 512, (n + 1) * 512
                        u = ps_u.tile([128, 512], F32, tag="u", name="u")
                        for k in range(KT):
                            nc.tensor.matmul(
                                u[:, :],
                                p_sb[qc][1][k][:, t * 128 : (t + 1) * 128],
                                v_sb[k][:, lo:hi],
                                start=(k == 0),
                                stop=(k == KT - 1),
                            )
                        o = ostage.tile([128, 512], F32, tag="o", name="o")
                        if qc == NQC - 1 and t == 3 and n == DIM // 512 - 1:
                            # final group: slice the drain so copy/DMA overlap
                            for sl in range(4):
                                s0, s1 = sl * 128, (sl + 1) * 128
                                if sl % 2 == 0:
                                    nc.scalar.copy(o[:, s0:s1], u[:, s0:s1])
                                else:
                                    nc.vector.tensor_copy(o[:, s0:s1], u[:, s0:s1])
                                nc.sync.dma_start(
                                    out=outp[row : row + 128, lo + s0 : lo + s1],
                                    in_=o[:, s0:s1],
                                )
                        else:
                            nc.scalar.copy(o[:, :], u[:, :])
                            nc.sync.dma_start(
                                out=outp[row : row + 128, lo:hi], in_=o[:, :]
                            )
                nc.leave_named_scope(f"attn_u{qc}", scope_u[0], False)

    return nc


_NC_CACHE = None


def _get_nc():
    global _NC_CACHE
    if _NC_CACHE is None:
        nc = _build_bass()
        fixed = _split_waits(bass.Bass.to_json_bytes(nc))
        nc.to_json_bytes = lambda: fixed
        _NC_CACHE = nc
    return _NC_CACHE


def kernel(hidden_states, W_q, b_q, W_k, b_k, W_v, b_v, scalar):
    global LAST_RESULTS
    bf16 = ml_dtypes.bfloat16
    X = np.asarray(hidden_states, np.float32)
    wqt = np.ascontiguousarray(np.asarray(W_q, np.float32).T).astype(bf16)
    wkt = np.ascontiguousarray(np.asarray(W_k, np.float32).T).astype(bf16)
    wvt = np.ascontiguousarray(np.asarray(W_v, np.float32).T).astype(bf16)
    bqr = np.ascontiguousarray(np.asarray(b_q, np.float32).reshape(CT, 128).T) * QKS
    bkr = np.ascontiguousarray(np.asarray(b_k, np.float32).reshape(CT, 128).T) * QKS
    bvb = np.ascontiguousarray(
        np.broadcast_to(np.asarray(b_v, np.float32), (128, DIM))
    )
    scv = np.full((128, 1), np.asarray(scalar, np.float32).reshape(-1)[0], np.float32)

    in_maps = []
    xts = {}
    for core in range(NCORES):
        b, h = core // 2, core % 2
        if b not in xts:
            xts[b] = np.ascontiguousarray(X[b].T).astype(bf16)
        xtq = np.ascontiguousarray(xts[b][:, h * QLEN : (h + 1) * QLEN])
        in_maps.append(
            {
                "xtq": xtq,
                "wqt": wqt,
                "wkt": wkt,
                "wvt": wvt,
                "bqr": bqr,
                "bkr": bkr,
                "bvb": bvb,
                "scv": scv,
            }
        )

    nc = _get_nc()
    res = run_bass_kernel_spmd(
        nc,
        in_maps,
        list(range(NCORES)),
        trace=TRACE,
    )
    LAST_RESULTS = res

    out = np.empty((B, S, DIM), np.float32)
    for core in range(NCORES):
        b, h = core // 2, core % 2
        out[b, h * QLEN : (h + 1) * QLEN, :] = res.results[core]["out"]
    return out


if __name__ == "__main__":
    import reference

    inputs = {k: np.asarray(v) for k, v in reference.setup_inputs().items()}
    got = kernel(**inputs)
    print("kernel output", got.shape, got.dtype)


# revision 43
# speedup vs baseline: 1.0702x; 1.0702x over previous
"""Trainium2 Bass kernel for nn_DiffAttn (differential attention).

Reference computation (per batch b):
    Q = X @ Wq.T + bq ; K = X @ Wk.T + bk ; V = X @ Wv.T + bv
    Q1,Q2 / K1,K2 = halves of feature dim
    A_j = (Q_j @ K_j.T) / sqrt(DIM)
    out = softmax(A1) @ V - scalar * softmax(A2) @ V

Sharding: 8 cores = 4 batches x 2 halves. Core (b,h) owns query rows
[h*1024,(h+1)*1024) of batch b AND computes the K^T/V projections only
for the SAME row block as keys; the two pair-mates exchange their K/V
halves with an intra-pair AllGather (replica groups {0,1},{2,3},...)
so neither projects the full sequence. Output slabs are disjoint.

Device-side layouts avoid all on-chip transposes: the host pre-transposes
X^T and W^T so every matmul contraction lands on SBUF partitions.
Q^T/K^T are produced in fp8e4 (scaled by QKS=4) directly by the
projection epilogue and the score matmuls run as fp8 DoubleRow (K=256
per instruction, 2x FLOP rate); P=exp and V are bf16 and the single
attn@V GEMM runs in bf16. Attention weights are normalized BEFORE the
V matmul (A = P1/r1 - scalar*P2/r2); row sums come from an all-ones
stationary matmul replicated across partitions, and 1/r is computed as
exp(-ln r) on the Scalar engine.

Overlap notes (measured on trn2, ~210 us HW exec, rel-err ~1.5e-2):
each AllGather is split in two chunks triggered mid-projection so the
~30 us mesh latency hides under the remaining phase-1 matmuls; both
score chunks run before either attn@V chunk so the serial
normalization tail overlaps score matmuls; weight DMAs are issued in
phase-consumption order (wk, wv, wq). Moving the row-sum accumulation
to GpSimd was tried and REGRESSED (gpsimd tensor ops ~1.6 us per
[128,512] tile, slower than the exp cadence).
"""

import json
import math
import os
from contextlib import ExitStack

import numpy as np
import ml_dtypes

import concourse.bass as bass
import concourse.tile as tile
from concourse import mybir
from concourse.bass_utils import run_bass_kernel_spmd


def _split_waits(raw: bytes, max_waits: int = 1) -> bytes:
    """walrus's CoreV3 codegen rejects instructions carrying more than one
    sync wait ("Too many sync wait commands"); Tile's kernel-tail drain
    aggregates one wait per live processor. Hoist excess waits onto chained
    same-engine Drain instructions inserted immediately before the offender."""
    m = json.loads(raw)
    uid = 0
    for fn in m["functions"]:
        for blk in fn["blocks"]:
            out = []
            for ins in blk["instructions"]:
                sy = ins.get("sync_info") or {}
                waits = sy.get("on_wait") or []
                if len(waits) > max_waits:
                    head, keep = waits[:-max_waits], waits[-max_waits:]
                    while head:
                        chunk, head = head[:max_waits], head[max_waits:]
                        uid += 1
                        out.append(
                            {
                                "engine": ins["engine"],
                                "ins": [],
                                "is_reset_sema": False,
                                "name": f"{ins['name']}-wsplit{uid}",
                                "opcode": "Drain",
                                "outs": [],
                                "sync_info": {"on_update": [], "on_wait": chunk},
                            }
                        )
                    sy["on_wait"] = keep
                out.append(ins)
            blk["instructions"] = out
    return json.dumps(m).encode()

B, S, DIM = 4, 2048, 1024
H = DIM // 2
NCORES = 8
QLEN = S // 2          # queries (and locally-projected keys) per core
SCALE = 1.0 / math.sqrt(DIM)

BF16 = mybir.dt.bfloat16
F32 = mybir.dt.float32
F32R = mybir.dt.float32r
FP8 = mybir.dt.float8e4
DR = mybir.MatmulPerfMode.DoubleRow

DT = DIM // 128        # 8  contraction tiles over model dim
CT = DIM // 128        # 8  feature tiles of Q^T/K^T
KT = S // 128          # 16 key tiles
KTL = QLEN // 128      # 8  local key tiles
NQC = QLEN // 512      # 2  query chunks of 512
VW = DIM              # V width (row sums come from an ones-row matmul instead)
QKS = 4.0             # fp8 pre-scale on Q^T/K^T (folded out of the exp scale)

# test harness hooks (the grader never touches these)
TRACE = False
LAST_RESULTS = None


def _build_bass():
    nc = bass.Bass(
        trn_type="TRN2",
        target_bir_lowering=False,
        debug=False,
        num_devices=NCORES,
    )

    xtq = nc.dram_tensor("xtq", [DIM, QLEN], BF16, kind="ExternalInput")
    wqt = nc.dram_tensor("wqt", [DIM, DIM], BF16, kind="ExternalInput")
    wkt = nc.dram_tensor("wkt", [DIM, DIM], BF16, kind="ExternalInput")
    wvt = nc.dram_tensor("wvt", [DIM, DIM], BF16, kind="ExternalInput")
    bqr = nc.dram_tensor("bqr", [128, CT], F32, kind="ExternalInput")
    bkr = nc.dram_tensor("bkr", [128, CT], F32, kind="ExternalInput")
    bvb = nc.dram_tensor("bvb", [128, DIM], F32, kind="ExternalInput")
    scv = nc.dram_tensor("scv", [128, 1], F32, kind="ExternalInput")
    outp = nc.dram_tensor("out", [QLEN, DIM], F32, kind="ExternalOutput")

    Id = mybir.ActivationFunctionType.Identity
    Exp = mybir.ActivationFunctionType.Exp
    PAIRS = [[0, 1], [2, 3], [4, 5], [6, 7]]

    with tile.TileContext(nc) as tc, ExitStack() as ctx:
        const = ctx.enter_context(tc.tile_pool(name="const", bufs=1))
        persist = ctx.enter_context(tc.tile_pool(name="persist", bufs=1))
        dram = ctx.enter_context(tc.tile_pool(name="dram", bufs=1, space="DRAM"))
        ps_s = ctx.enter_context(
            tc.tile_pool(name="ps_s", bufs=3, space="PSUM")
        )

        bq_sb = const.tile([128, CT], F32)
        nc.sync.dma_start(out=bq_sb[:, :], in_=bqr[:, :])
        bk_sb = const.tile([128, CT], F32)
        nc.sync.dma_start(out=bk_sb[:, :], in_=bkr[:, :])
        sc_sb = const.tile([128, 1], F32)
        nc.sync.dma_start(out=sc_sb[:, :], in_=scv[:, :])
        ones_sb = const.tile([128, 2], F32)
        nc.vector.memset(ones_sb[:, :], 1.0)

        # Warm the PE clock gate (HAM) during the initial input-DMA wait:
        # a chain of tiny dependent matmuls gives sustained PE activity so
        # the first projection matmuls run at 2.4 GHz, not 1.2.
        with tc.psum_pool(name="ps_w", bufs=1) as ps_w:
            warm = ps_w.tile([2, 2], F32, name="warm")
            for _ in range(24):
                nc.tensor.matmul(
                    warm[:, :], ones_sb[:, :], ones_sb[:, :], start=True, stop=True
                )

        # persistent products of the projection phase.
        # Q^T/K^T are fp8e4 (scaled by QKS) in DoubleRow-paired tiles:
        # tile t holds feature rows (256t..256t+128) in slot 0 and
        # (256t+128..256t+256) in slot 1 so a [128, 2, n] slice is a K=256
        # fp8 DoubleRow operand. V is bf16.
        q_sb = [persist.tile([128, 2, QLEN], FP8, name=f"q{i}") for i in range(CT // 2)]
        k_sb = [persist.tile([128, 2, S], FP8, name=f"k{i}") for i in range(CT // 2)]
        v_sb = [persist.tile([128, VW], BF16, name=f"v{i}") for i in range(KT)]

        # AllGather bounce buffers (flat concat along the leading axis:
        # out.reshape(2, ...)[g] = rank g's input within the pair).
        # Both gathers are split in two so the wire transfer starts while
        # the second half of the projection is still running.
        kag_in = [
            dram.tile([2, 128, 2, QLEN], FP8, name=f"kag_in{i}") for i in range(2)
        ]
        kag_out = [
            dram.tile([2, 2, 128, 2, QLEN], FP8, name=f"kag_out{i}") for i in range(2)
        ]
        vag_in = [
            dram.tile([KTL // 2, 128, VW], BF16, name=f"vag_in{i}") for i in range(2)
        ]
        vag_out = [
            dram.tile([2, KTL // 2, 128, VW], BF16, name=f"vag_out{i}")
            for i in range(2)
        ]

        # X^T tiles for this core's row block: queries for Q^T, and the
        # same rows serve as the locally-projected keys for K^T/V.
        xtp = tc.alloc_tile_pool(name="xtp", bufs=1)
        xq_t = [xtp.tile([128, QLEN], BF16, name=f"xq{d}") for d in range(DT)]

        # weight pools: wk / wv / wq — DMAs issued in phase-consumption order
        # so the DMA queues deliver each phase's operands just ahead of its
        # matmuls (wv before wq: phase 1b precedes 1c).
        wkp = tc.alloc_tile_pool(name="wk", bufs=1)
        wvp = tc.alloc_tile_pool(name="wv", bufs=1)
        wqp = tc.alloc_tile_pool(name="wq", bufs=1)
        wk_t = [wkp.tile([128, DIM], BF16, name=f"wk{d}") for d in range(DT)]
        wv_t = [wvp.tile([128, DIM], BF16, name=f"wv{d}") for d in range(DT)]
        bv_sb = wvp.tile([128, DIM], F32, name="bv_sb")
        wq_t = [wqp.tile([128, DIM], BF16, name=f"wq{d}") for d in range(DT)]
        # First-wave DMAs are column-halved so the opening Kproj block
        # (c 0..3, n=0) only waits on 2MB (xq/wk first halves), not 4MB.
        for d in range(DT):
            nc.sync.dma_start(
                out=xq_t[d][:, :512], in_=xtq[d * 128 : (d + 1) * 128, :512]
            )
            nc.sync.dma_start(
                out=wk_t[d][:, :512], in_=wkt[d * 128 : (d + 1) * 128, :512]
            )
        for d in range(DT):
            nc.sync.dma_start(
                out=xq_t[d][:, 512:], in_=xtq[d * 128 : (d + 1) * 128, 512:]
            )
        for d in range(DT):
            nc.sync.dma_start(
                out=wk_t[d][:, 512:], in_=wkt[d * 128 : (d + 1) * 128, 512:]
            )
        nc.sync.dma_start(out=bv_sb[:, :], in_=bvb[:, :])
        for d in range(DT):
            nc.sync.dma_start(out=wv_t[d][:, :], in_=wvt[d * 128 : (d + 1) * 128, :])
        for d in range(DT):
            nc.sync.dma_start(out=wq_t[d][:, :], in_=wqt[d * 128 : (d + 1) * 128, :])

        # ---- Phase 1a: local K^T half = Wk^T.T @ X^T[:, own rows]  (+bk) ----
        # The pair exchange is split in two chunks (feature tiles {0,1} then
        # {2,3}), each triggered as soon as its projection slice is done, so
        # the mesh latency overlaps the rest of phase 1.
        kh = [persist.tile([128, 2, QLEN], FP8, name=f"kh{i}") for i in range(CT // 2)]
        with nc.named_scope("proj_k"):
            # (c-group, n) block order so the first block only touches the
            # first-wave DMA halves; each c-group's AllGather chunk fires as
            # soon as both its n chunks are done.
            korder = [
                (cg, n) for cg in range(2) for n in range(QLEN // 512)
            ]
            for cg, n in korder:
                for c in range(4 * cg, 4 * cg + 4):
                    ps = ps_s.tile([128, 512], F32, tag="ps", name="psk")
                    for d in range(DT):
                        nc.tensor.matmul(
                            ps[:, :],
                            wk_t[d][:, c * 128 : (c + 1) * 128],
                            xq_t[d][:, n * 512 : (n + 1) * 512],
                            start=(d == 0),
                            stop=(d == DT - 1),
                        )
                    nc.scalar.activation(
                        kh[c // 2][:, c % 2, n * 512 : (n + 1) * 512],
                        ps[:, :],
                        Id,
                        scale=QKS,
                        bias=bk_sb[:, c : c + 1],
                    )
                if n == QLEN // 512 - 1:
                    ch = cg
                    for tt in range(2):
                        t = 2 * ch + tt
                        nc.sync.dma_start(
                            out=kag_in[ch][tt, :, :, :], in_=kh[t][:, :, :]
                        )
                    nc.gpsimd.collective_compute(
                        "AllGather",
                        mybir.AluOpType.bypass,
                        replica_groups=PAIRS,
                        ins=[kag_in[ch].opt()],
                        outs=[kag_out[ch].opt()],
                    )
                    for hh in range(2):
                        for tt in range(2):
                            t = 2 * ch + tt
                            nc.sync.dma_start(
                                out=k_sb[t][:, :, hh * QLEN : (hh + 1) * QLEN],
                                in_=kag_out[ch][hh, tt, :, :, :],
                            )

        # ---- Phase 1b: Q^T = Wq^T.T @ X^T[:, own rows]  (+bq) ----
        with nc.named_scope("proj_q"):
            for c in range(CT):
                for n in range(QLEN // 512):
                    ps = ps_s.tile([128, 512], F32, tag="ps", name="psq")
                    for d in range(DT):
                        nc.tensor.matmul(
                            ps[:, :],
                            wq_t[d][:, c * 128 : (c + 1) * 128],
                            xq_t[d][:, n * 512 : (n + 1) * 512],
                            start=(d == 0),
                            stop=(d == DT - 1),
                        )
                    nc.scalar.activation(
                        q_sb[c // 2][:, c % 2, n * 512 : (n + 1) * 512],
                        ps[:, :],
                        Id,
                        scale=QKS,
                        bias=bq_sb[:, c : c + 1],
                    )

        # ---- Phase 1c: local V half = X^T[:, own rows].T @ Wv^T (+bv) ----
        vh = [persist.tile([128, VW], BF16, name=f"vh{i}") for i in range(KTL)]
        with nc.named_scope("proj_v"):
            for k in range(KTL):
                for n in range(DIM // 512):
                    ps = ps_s.tile([128, 512], F32, tag="ps", name="psv")
                    for d in range(DT):
                        nc.tensor.matmul(
                            ps[:, :],
                            xq_t[d][:, k * 128 : (k + 1) * 128],
                            wv_t[d][:, n * 512 : (n + 1) * 512],
                            start=(d == 0),
                            stop=(d == DT - 1),
                        )
                    nc.vector.tensor_add(
                        vh[k][:, n * 512 : (n + 1) * 512],
                        ps[:, :],
                        bv_sb[:, n * 512 : (n + 1) * 512],
                    )
                if k % 4 == 3:
                    ch = k // 4
                    for kk in range(4):
                        nc.sync.dma_start(
                            out=vag_in[ch][kk, :, :], in_=vh[4 * ch + kk][:, :]
                        )
                    nc.gpsimd.collective_compute(
                        "AllGather",
                        mybir.AluOpType.bypass,
                        replica_groups=PAIRS,
                        ins=[vag_in[ch].opt()],
                        outs=[vag_out[ch].opt()],
                    )
                    for hh in range(2):
                        for kk in range(4):
                            nc.sync.dma_start(
                                out=v_sb[hh * KTL + 4 * ch + kk][:, :],
                                in_=vag_out[ch][hh, kk, :, :],
                            )

        wqp.release()
        wvp.release()
        wkp.release()
        xtp.release()

        # ---- Phase 2: attention, one 512-query chunk at a time ----
        # Normalize P before the V matmul so only ONE attn@V GEMM is needed:
        #   A^T = P1^T * bcast(1/r1) - P2^T * bcast(scalar/r2);  out = A^T.T @ V
        # r_j comes from an ones-row stationary matmul (column sums of P^T);
        # bcast replicates the [1, q] reciprocal row across partitions via a
        # K=1 ones-column matmul.
        lnsc_sb = const.tile([128, 1], F32)
        nc.scalar.activation(lnsc_sb[:, :], sc_sb[:, :], mybir.ActivationFunctionType.Ln)
        ones_sq = const.tile([128, 128], BF16)
        nc.vector.memset(ones_sq[:, :], 1.0)

        with (
            tc.tile_pool(name="pP", bufs=1) as pP,
            tc.tile_pool(name="ps_r", bufs=2, space="PSUM") as ps_r,
            tc.tile_pool(name="ps_u", bufs=3, space="PSUM") as ps_u,
            tc.tile_pool(name="small", bufs=4) as small,
            tc.tile_pool(name="tmp2", bufs=2) as tmp2,
            tc.tile_pool(name="ostage", bufs=3) as ostage,
        ):
            p_sb = [
                [
                    [pP.tile([128, 512], BF16, name=f"p{qc}_{j}_{k}") for k in range(KT)]
                    for j in range(2)
                ]
                for qc in range(NQC)
            ]
            # Both scores chunks run back-to-back on the PE so the serial
            # normalization tail (rowsum -> ln -> exp -> mul/sub) of chunk
            # qc overlaps the next chunk's score matmuls; attn@V follows.
            for qc in range(NQC):
                # scores S^T[k, q] = K_j^T.T @ Q_j^T via fp8 DoubleRow (K=512
                # in 2 accumulating K=256 MMs); P = exp(s*S^T); r = col sums
                bcs = []
                scope_s = nc.enter_named_scope(f"attn_s{qc}", False)
                for j in range(2):
                    # r replicated across partitions: ones[128,128].T @ P = col sums
                    r_ps = ps_r.tile([128, 512], F32, tag="r", name=f"r{j}")
                    for k in range(KT):
                        ps = ps_s.tile([128, 512], F32, tag="ps", name="pss")
                        for ti in range(2):
                            t = 2 * j + ti
                            nc.tensor.matmul(
                                ps[:, :],
                                k_sb[t][:, :, k * 128 : (k + 1) * 128],
                                q_sb[t][:, :, qc * 512 : (qc + 1) * 512],
                                start=(ti == 0),
                                stop=(ti == 1),
                                perf_mode=DR,
                            )
                        nc.scalar.activation(
                            p_sb[qc][j][k][:, :], ps[:, :], Exp, scale=SCALE / (QKS * QKS)
                        )
                        nc.tensor.matmul(
                            r_ps[:, :],
                            ones_sq[:, :],
                            p_sb[qc][j][k][:, :],
                            start=(k == 0),
                            stop=(k == KT - 1),
                        )
                    # bc_j = exp(-ln r_j) = 1/r_j on the Scalar engine
                    # (j=1 folds the input scalar in via a +ln(scalar) bias)
                    lnr = tmp2.tile([128, 512], F32, tag="lnr", name="lnr")
                    nc.scalar.activation(
                        lnr[:, :], r_ps[:, :], mybir.ActivationFunctionType.Ln
                    )
                    bc = small.tile([128, 512], F32, tag=f"bc{j}", name=f"bc{j}")
                    if j == 0:
                        nc.scalar.activation(bc[:, :], lnr[:, :], Exp, scale=-1.0)
                    else:
                        nc.scalar.activation(
                            bc[:, :], lnr[:, :], Exp, scale=-1.0, bias=lnsc_sb[:, :]
                        )
                    bcs.append(bc)
                    if j == 0:
                        # P1 *= 1/r1 immediately — overlaps the j=1 scores
                        for k in range(KT):
                            nc.vector.tensor_mul(
                                p_sb[qc][0][k][:, :], p_sb[qc][0][k][:, :], bc[:, :]
                            )
                nc.leave_named_scope(f"attn_s{qc}", scope_s[0], False)

                # A^T[k] = P1[k] - P2[k]*bc2s  (in place into p_sb[qc][1])
                scope_a = nc.enter_named_scope(f"attn_a{qc}", False)
                for k in range(KT):
                    nc.vector.tensor_mul(
                        p_sb[qc][1][k][:, :], p_sb[qc][1][k][:, :], bcs[1][:, :]
                    )
                    nc.vector.tensor_sub(
                        p_sb[qc][1][k][:, :], p_sb[qc][0][k][:, :], p_sb[qc][1][k][:, :]
                    )
                nc.leave_named_scope(f"attn_a{qc}", scope_a[0], False)

            # out rows = A^T.T @ V (both chunks)
            for qc in range(NQC):
                scope_u = nc.enter_named_scope(f"attn_u{qc}", False)
                for t in range(4):
                    row = qc * 512 + t * 128
                    for n in range(DIM // 512):
                        lo, hi = n * 512, (n + 1) * 512
                        u = ps_u.tile([128, 512], F32, tag="u", name="u")
                        for k in range(KT):
                            nc.tensor.matmul(
                                u[:, :],
                                p_sb[qc][1][k][:, t * 128 : (t + 1) * 128],
                                v_sb[k][:, lo:hi],
                                start=(k == 0),
                                stop=(k == KT - 1),
                            )
                        o = ostage.tile([128, 512], F32, tag="o", name="o")
                        nc.scalar.copy(o[:, :], u[:, :])
                        nc.sync.dma_start(
                            out=outp[row : row + 128, lo:hi], in_=o[:, :]
                        )
                nc.leave_named_scope(f"attn_u{qc}", scope_u[0], False)

    return nc


_NC_CACHE = None


def _get_nc():
    global _NC_CACHE
    if _NC_CACHE is None:
        nc = _build_bass()
        fixed = _split_waits(bass.Bass.to_json_bytes(nc))
        nc.to_json_bytes = lambda: fixed
        _NC_CACHE = nc
    return _NC_CACHE


def kernel(hidden_states, W_q, b_q, W_k, b_k, W_v, b_v, scalar):
    global LAST_RESULTS
    bf16 = ml_dtypes.bfloat16
    X = np.asarray(hidden_states, np.float32)
    wqt = np.ascontiguousarray(np.asarray(W_q, np.float32).T).astype(bf16)
    wkt = np.ascontiguousarray(np.asarray(W_k, np.float32).T).astype(bf16)
    wvt = np.ascontiguousarray(np.asarray(W_v, np.float32).T).astype(bf16)
    bqr = np.ascontiguousarray(np.asarray(b_q, np.float32).reshape(CT, 128).T) * QKS
    bkr = np.ascontiguousarray(np.asarray(b_k, np.float32).reshape(CT, 128).T) * QKS
    bvb = np.ascontiguousarray(
        np.broadcast_to(np.asarray(b_v, np.float32), (128, DIM))
    )
    scv = np.full((128, 1), np.asarray(scalar, np.float32).reshape(-1)[0], np.float32)

    in_maps = []
    xts = {}
    for core in range(NCORES):
        b, h = core // 2, core % 2
        if b not in xts:
            xts[b] = np.ascontiguousarray(X[b].T).astype(bf16)
        xtq = np.ascontiguousarray(xts[b][:, h * QLEN : (h + 1) * QLEN])
        in_maps.append(
            {
                "xtq": xtq,
                "wqt": wqt,
                "wkt": wkt,
                "wvt": wvt,
                "bqr": bqr,
                "bkr": bkr,
                "bvb": bvb,
                "scv": scv,
            }
        )

    nc = _get_nc()
    res = run_bass_kernel_spmd(
        nc,
        in_maps,
        list(range(NCORES)),
        trace=TRACE,
    )
    LAST_RESULTS = res

    out = np.empty((B, S, DIM), np.float32)
    for core in range(NCORES):
        b, h = core // 2, core % 2
        out[b, h * QLEN : (h + 1) * QLEN, :] = res.results[core]["out"]
    return out


if __name__ == "__main__":
    import reference

    inputs = {k: np.asarray(v) for k, v in reference.setup_inputs().items()}
    got = kernel(**inputs)
    print("kernel output", got.shape, got.dtype)
